# revision 9
# baseline (speedup 1.0000x reference)
"""Trainium2 Bass kernel for a 2-layer hetero RGAT (DGL-style), 8 NeuronCores.

Sharding: edges dst-sorted, sharded by contiguous 2560-dst ranges per core
(edge/graph partition parallelism, per-dst segment sums stay core-local; the
only collective is an AllGather of the small layer-1 node features).

Per conv on device:
  table = x_src @ [W | W.al]  -> DRAM [20480, 384] bf16 rows [fs(256)|el(8)|pad]
  er    = x_dst_local @ (W.ar)              (per-core local dst blocks)
  SWDGE dma_gather of 768B table rows by u (slices of 40 chunks x 128 edges)
  per 128-edge chunk:
    er_edge = P01T.T @ er_block             (PE, host-built fp8 0/1 one-hot)
    l = el + er_edge ; w = exp(max(l, .2l)) (DVE + ACT)  == exp(leaky_relu(l))
    G[:, :256] *= broadcast_D(w)            (DVE bf16, in-place)
    psum[block] += P01.T @ G[:, :264]       (PE fp8 one-hot lhsT; cols 256:264=w)
  per 128-dst block: acc += psum[:, :256] / max(psum[:, 256:264], eps)
Softmax max-subtraction is skipped (exactly equivalent; no overflow at these
scales).  h' = relu(sum_h(acc) + 8*mean_h(bias)) = 8*h with the 1/8 folded
into the layer-2 weights; final outputs scaled by 1/8 at the end.
"""

import sys
import numpy as np
import ml_dtypes

sys.path.insert(0, "/opt/trn_rl_repo")

F16 = np.float16
FP8 = ml_dtypes.float8_e4m3

FIN = 256
H, D = 8, 32
HD = H * D
TW = 384
ELOFF = HD
NCORE = 8
BLK = 128
NEG = 0.2
DEN_EPS = 1e-30


def configure(n=20000, nblk=20, sl=40):
    """Set problem scale (module globals). Default = full problem."""
    global N, NBLK, DPC, TN, NT1, NP1, NT2, SL
    N = n
    NBLK = nblk
    DPC = NBLK * BLK
    TN = DPC * NCORE
    NT1 = (N + BLK - 1) // BLK
    NP1 = NT1 * BLK
    NT2 = TN // BLK
    SL = sl


configure()


def _prep_graph(u, v):
    """Sort by dst, shard to cores by 2560-dst ranges, pad blocks to 128."""
    order = np.argsort(v, kind="stable")
    us = u[order].astype(np.int64)
    vs = v[order].astype(np.int64)
    core_of = vs // DPC
    K = 1
    core_data = []
    for c in range(NCORE):
        sel = core_of == c
        uc = us[sel]
        vc = vs[sel] - c * DPC
        blk = vc // BLK
        cnt = np.bincount(blk, minlength=NBLK)
        K = max(K, int(np.ceil(cnt.max() / BLK)))
        core_data.append((uc, vc, cnt))
    out = []
    nch = NBLK * K
    for c in range(NCORE):
        uc, vc, cnt = core_data[c]
        ebase = np.concatenate([[0], np.cumsum(cnt)])
        u_pad = np.zeros(nch * BLK, np.int32)  # pad edges: any row; P01 col is zero
        vloc = np.zeros(nch * BLK, np.int32)
        valid = np.zeros(nch * BLK, np.bool_)
        for b in range(NBLK):
            n = int(cnt[b])
            dst = b * K * BLK
            u_pad[dst:dst + n] = uc[ebase[b]:ebase[b + 1]]
            vloc[dst:dst + n] = vc[ebase[b]:ebase[b + 1]] - b * BLK
            valid[dst:dst + n] = True
        p01 = np.zeros((nch, BLK, BLK), np.uint8)
        p01t = np.zeros((nch, BLK, BLK), np.uint8)
        idx = np.arange(nch * BLK)
        ch, row = idx[valid] // BLK, idx[valid] % BLK
        col = vloc[valid]
        p01[ch, row, col] = 0x38          # 1.0 in fp8e4m3
        p01t[ch, col, row] = 0x38
        u16 = np.tile(u_pad.astype(np.int16).reshape(-1, 16).T, (8, 1)).copy()
        out.append(dict(
            u16=np.ascontiguousarray(u16),                       # [128, nch*8]
            p01=p01.reshape(nch * BLK, BLK).view(FP8),
            p01t=p01t.reshape(nch * BLK, BLK).view(FP8),
        ))
    return K, out


def _fold(W, a):
    return np.einsum("ihd,hd->ih", W.reshape(W.shape[0], H, D), a)


def _build_program(KS):
    import concourse.bacc as bacc
    import concourse.mybir as mybir
    import concourse.tile as tile

    dt = mybir.dt
    nc = bacc.Bacc("TRN2", target_bir_lowering=False, debug=False,
                   num_devices=NCORE)

    NCH = [NBLK * k for k in KS]

    P = nc.declare_dram_parameter
    xAT = P("xAT", [FIN, NP1], dt.float16, isOutput=False)
    xBT = P("xBT", [FIN, NP1], dt.float16, isOutput=False)
    xLA = P("xLA", [FIN, DPC], dt.float16, isOutput=False)
    xLB = P("xLB", [FIN, DPC], dt.float16, isOutput=False)
    w1 = P("w1", [3, FIN, HD + H], dt.float16, isOutput=False)
    wr1 = P("wr1", [3, FIN, H], dt.float16, isOutput=False)
    w2 = P("w2", [3, D, HD + H], dt.float16, isOutput=False)
    wr2 = P("wr2", [3, D, H], dt.float16, isOutput=False)
    bm = P("bm", [4, BLK, D], dt.float32, isOutput=False)
    ident = P("ident", [BLK, BLK], dt.float16, isOutput=False)
    u16 = [P(f"u16_{g}", [BLK, NCH[g] * 8], dt.int16, isOutput=False)
           for g in range(3)]
    p01 = [P(f"p01_{g}", [NCH[g] * BLK, BLK], dt.float8e4, isOutput=False)
           for g in range(3)]
    p01t = [P(f"p01t_{g}", [NCH[g] * BLK, BLK], dt.float8e4, isOutput=False)
            for g in range(3)]
    oA = P("oA", [DPC, D], dt.float32, isOutput=True)
    oB = P("oB", [DPC, D], dt.float32, isOutput=True)

    tabs = [nc.dram_tensor(f"table{i}", [TN, TW], dt.float16) for i in range(2)]
    hT_loc = [nc.dram_tensor(f"hT{s}_loc", [D, DPC], dt.float16)
              for s in range(2)]
    hT_full = [nc.dram_tensor(f"hT{s}_full", [NCORE * D, DPC], dt.float16,
                              addr_space="Shared") for s in range(2)]

    with tile.TileContext(nc) as tc:
        from contextlib import ExitStack
        with ExitStack() as es:
            cpool = es.enter_context(tc.tile_pool(name="consts", bufs=1))
            wpool = es.enter_context(tc.tile_pool(name="wts", bufs=2))
            xpool = es.enter_context(tc.tile_pool(name="xt", bufs=4))
            tspool = es.enter_context(tc.tile_pool(name="tsb", bufs=4))
            gpool = es.enter_context(tc.tile_pool(name="g", bufs=2))
            ppool = es.enter_context(tc.tile_pool(name="p01", bufs=2))
            upool = es.enter_context(tc.tile_pool(name="u16", bufs=2))
            lpool = es.enter_context(tc.tile_pool(name="l", bufs=2))
            wbpool = es.enter_context(tc.tile_pool(name="wb", bufs=1))
            erpool = es.enter_context(tc.tile_pool(name="er", bufs=2))
            ndpool = es.enter_context(tc.tile_pool(name="nd", bufs=3))
            accpool = es.enter_context(tc.tile_pool(name="acc", bufs=1))
            hpool = es.enter_context(tc.tile_pool(name="h", bufs=1))
            mpool = es.enter_context(tc.tile_pool(name="misc", bufs=2))
            xlpool = es.enter_context(tc.tile_pool(name="xl", bufs=1))
            ps_proj = es.enter_context(tc.tile_pool(name="ps_p", bufs=2, space="PSUM"))
            ps_agg = es.enter_context(tc.tile_pool(name="ps_a", bufs=2, space="PSUM"))
            ps_ere = es.enter_context(tc.tile_pool(name="ps_e", bufs=2, space="PSUM"))
            ps_erp = es.enter_context(tc.tile_pool(name="ps_r", bufs=1, space="PSUM"))
            ps_tr = es.enter_context(tc.tile_pool(name="ps_t", bufs=1, space="PSUM"))

            accA = accpool.tile([BLK, NBLK, HD], dt.float32, tag="accA")
            accB = accpool.tile([BLK, NBLK, HD], dt.float32, tag="accB")
            bm_sb = cpool.tile([BLK, 4, D], dt.float32, tag="bm")
            nc.sync.dma_start(bm_sb[:], bm[:, :, :].rearrange("b p d -> p b d"))
            id_sb = cpool.tile([BLK, BLK], dt.float16, tag="id")
            nc.sync.dma_start(id_sb[:], ident[:, :])

            zt = cpool.tile([BLK, TW], dt.float16, tag="zt")
            nc.vector.memset(zt[:], 0.0)
            for t in range(2):
                for r0 in range(NP1, TN, BLK):
                    nc.sync.dma_start(tabs[t][r0:r0 + BLK, :], zt[:])

            def load_w(wsrc, e, kdim):
                ks = 2 if kdim > BLK else 1
                kp = BLK if ks == 2 else kdim
                wt = wpool.tile([BLK, 2, HD + H], dt.float16, tag="wmain")
                if ks == 2:
                    nc.sync.dma_start(
                        wt[:], wsrc[e].rearrange("(a k) o -> k a o", k=BLK))
                else:
                    nc.sync.dma_start(wt[:kp, 0, :], wsrc[e])
                return wt, ks, kp

            def load_wr(wrsrc, e, kdim):
                ks = 2 if kdim > BLK else 1
                kp = BLK if ks == 2 else kdim
                wt = wpool.tile([BLK, 2, H], dt.float16, tag="wr")
                if ks == 2:
                    nc.sync.dma_start(
                        wt[:], wrsrc[e].rearrange("(a k) o -> k a o", k=BLK))
                else:
                    nc.sync.dma_start(wt[:kp, 0, :], wrsrc[e])
                return wt, ks, kp

            def projection(srcT_dram, stacked_dram, wt, ks, kp, table, ntiles):
                for t in range(ntiles):
                    lhs = xpool.tile([BLK, 2, BLK], dt.float16, tag="lhs")
                    if srcT_dram is not None:
                        nc.sync.dma_start(
                            lhs[:kp, :ks, :],
                            srcT_dram[:, t * BLK:(t + 1) * BLK]
                            .rearrange("(a k) n -> k a n", k=kp))
                    else:
                        c, bt = t // NBLK, t % NBLK
                        nc.sync.dma_start(
                            lhs[:kp, 0, :],
                            stacked_dram[c * D:(c + 1) * D,
                                         bt * BLK:(bt + 1) * BLK])
                    lhs_aps = [lhs[:kp, a, :] for a in range(ks)]
                    ps = ps_proj.tile([BLK, HD + H], dt.float32, tag="pp")
                    for a in range(ks):
                        nc.tensor.matmul(ps[:], lhs_aps[a], wt[:kp, a, :],
                                         start=(a == 0), stop=(a == ks - 1))
                    tsb = tspool.tile([BLK, HD + H], dt.float16, tag="tsb")
                    if t % 2 == 0:
                        nc.scalar.copy(tsb[:], ps[:])
                    else:
                        nc.vector.tensor_copy(tsb[:], ps[:])
                    nc.sync.dma_start(table[t * BLK:(t + 1) * BLK, :HD + H],
                                      tsb[:])

            def er_projection(dstT_dram, dstT_sb, wrt, ks, kp):
                if dstT_dram is not None:
                    xl = xlpool.tile([BLK, 2, DPC], dt.float16, tag="xl")
                    nc.sync.dma_start(
                        xl[:kp, :ks, :],
                        dstT_dram[:, :].rearrange("(a k) n -> k a n", k=kp))
                ps = ps_erp.tile([BLK, NBLK * H], dt.float32, tag="erp")
                for b in range(NBLK):
                    for a in range(ks):
                        if dstT_dram is not None:
                            lhs = xl[:kp, a, b * BLK:(b + 1) * BLK]
                        else:
                            lhs = dstT_sb[:kp, b * BLK:(b + 1) * BLK]
                        nc.tensor.matmul(ps[:, b * H:(b + 1) * H], lhs,
                                         wrt[:kp, a, :],
                                         start=(a == 0), stop=(a == ks - 1))
                er_sb = erpool.tile([BLK, NBLK * H], dt.float16, tag="ersb")
                nc.scalar.copy(er_sb[:], ps[:])
                return er_sb

            def tail(psb, b, acc, first):
                nd = ndpool.tile([BLK, HD + H], dt.float32, tag="nd")
                nc.scalar.copy(nd[:], psb[:])
                rec = mpool.tile([BLK, H], dt.float32, tag="rec")
                nc.vector.tensor_scalar(rec[:], nd[:, HD:HD + H], DEN_EPS, None,
                                        mybir.AluOpType.max)
                nc.vector.reciprocal(rec[:], rec[:])
                recb = rec[:].unsqueeze(2).broadcast_to([BLK, H, D])
                if first:
                    nc.vector.tensor_tensor(
                        acc[:, b, :].rearrange("p (h d) -> p h d", h=H),
                        nd[:, :HD].rearrange("p (h d) -> p h d", h=H),
                        recb, mybir.AluOpType.mult)
                else:
                    tmp = mpool.tile([BLK, HD], dt.float32, tag="tmp")
                    nc.vector.tensor_tensor(
                        tmp[:].rearrange("p (h d) -> p h d", h=H),
                        nd[:, :HD].rearrange("p (h d) -> p h d", h=H),
                        recb, mybir.AluOpType.mult)
                    nc.vector.tensor_tensor(acc[:, b, :], acc[:, b, :], tmp[:],
                                            mybir.AluOpType.add)

            def conv(g, table, er_sb, acc, first):
                K = KS[g]
                nch = NCH[g]
                nslice = (nch + SL - 1) // SL
                psb = None
                for s in range(nslice):
                    c0 = s * SL
                    ns = min(SL, nch - c0)
                    u_sb = upool.tile([BLK, SL * 8], dt.int16, tag="usb")
                    nc.sync.dma_start(u_sb[:, :ns * 8],
                                      u16[g][:, c0 * 8:(c0 + ns) * 8])
                    G = gpool.tile([BLK, SL, TW], dt.float16, tag="G")
                    GSUB = 8  # chunks per dma_gather (SWDGE ring cap ~1-2k descs)
                    for j0 in range(0, ns, GSUB):
                        nj = min(GSUB, ns - j0)
                        nc.gpsimd.dma_gather(
                            G[:, j0:j0 + nj, :], table[:, :],
                            u_sb[:, j0 * 8:(j0 + nj) * 8],
                            num_idxs=nj * BLK, num_idxs_reg=nj * BLK,
                            elem_size=TW)
                    pt = ppool.tile([BLK, 2, SL, BLK], dt.float8e4, tag="p01")
                    nc.sync.dma_start(
                        pt[:, 0, :ns, :],
                        p01[g][c0 * BLK:(c0 + ns) * BLK, :]
                        .rearrange("(c e) x -> e c x", e=BLK))
                    nc.sync.dma_start(
                        pt[:, 1, :ns, :],
                        p01t[g][c0 * BLK:(c0 + ns) * BLK, :]
                        .rearrange("(c e) x -> e c x", e=BLK))
                    pse = ps_ere.tile([BLK, SL * H], dt.float32, tag="pse")
                    for j in range(ns):
                        b = (c0 + j) // K
                        nc.tensor.matmul(
                            pse[:, j * H:(j + 1) * H], pt[:, 1, j, :],
                            er_sb[:, b * H:(b + 1) * H], start=True, stop=True)
                    lt = lpool.tile([BLK, SL * H], dt.float16, tag="lt")
                    nc.vector.tensor_tensor(
                        lt[:, :ns * H].rearrange("p (c h) -> p c h", h=H),
                        G[:, :ns, ELOFF:ELOFF + H],
                        pse[:, :ns * H].rearrange("p (c h) -> p c h", h=H),
                        mybir.AluOpType.add)
                    l5 = lpool.tile([BLK, SL * H], dt.float16, tag="l5")
                    nc.vector.tensor_scalar(l5[:, :ns * H], lt[:, :ns * H],
                                            NEG, None, mybir.AluOpType.mult)
                    l2 = lpool.tile([BLK, SL * H], dt.float16, tag="l2")
                    nc.vector.tensor_tensor(l2[:, :ns * H], lt[:, :ns * H],
                                            l5[:, :ns * H], mybir.AluOpType.max)
                    nc.scalar.activation(
                        G[:, :ns, ELOFF:ELOFF + H],
                        l2[:, :ns * H].rearrange("p (c h) -> p c h", h=H),
                        mybir.ActivationFunctionType.Exp)
                    wb = wbpool.tile([BLK, SL, HD], dt.float16, tag="wb")
                    nc.gpsimd.tensor_copy(
                        wb[:, :ns, :].rearrange("p c (h d) -> p c h d", h=H),
                        G[:, :ns, ELOFF:ELOFF + H].unsqueeze(3)
                        .broadcast_to([BLK, ns, H, D]))
                    nc.vector.tensor_tensor(G[:, :ns, :HD], G[:, :ns, :HD],
                                            wb[:, :ns, :],
                                            mybir.AluOpType.mult)
                    for j in range(ns):
                        jj = c0 + j
                        b = jj // K
                        if jj % K == 0:
                            psb = ps_agg.tile([BLK, HD + H], dt.float32,
                                              tag="psb")
                        nc.tensor.matmul(psb[:], pt[:, 0, j, :],
                                         G[:, j, :HD + H],
                                         start=(jj % K == 0),
                                         stop=(jj % K == K - 1))
                        if jj % K == K - 1:
                            tail(psb, b, acc, first)

            def hstage(acc, bmi, relu, hT_loc_dram):
                hs = hpool.tile([BLK, NBLK, D], dt.float32, tag=f"hs{bmi}")
                nc.vector.tensor_reduce(
                    hs[:], acc[:].rearrange("p b (h d) -> p b d h", h=H),
                    mybir.AxisListType.X, mybir.AluOpType.add)
                nc.vector.tensor_tensor(
                    hs[:], hs[:],
                    bm_sb[:, bmi, :].unsqueeze(1).broadcast_to([BLK, NBLK, D]),
                    mybir.AluOpType.add)
                hb = hpool.tile([BLK, NBLK, D], dt.float16, tag=f"hb{bmi}")
                if relu:
                    nc.scalar.activation(hb[:], hs[:],
                                         mybir.ActivationFunctionType.Relu)
                else:
                    nc.scalar.copy(hb[:], hs[:])
                hT = hpool.tile([D, DPC], dt.float16, tag=f"hT{bmi}")
                for b in range(NBLK):
                    ptr = ps_tr.tile([D, BLK], dt.float16, tag="ptr")
                    nc.tensor.transpose(ptr[:], hb[:, b, :], id_sb[:])
                    nc.scalar.copy(hT[:, b * BLK:(b + 1) * BLK], ptr[:])
                nc.sync.dma_start(hT_loc_dram[:, :], hT[:])
                return hT

            def ostage(acc, bmi, out_dram):
                hs = hpool.tile([BLK, NBLK, D], dt.float32, tag="hs0")
                nc.vector.tensor_reduce(
                    hs[:], acc[:].rearrange("p b (h d) -> p b d h", h=H),
                    mybir.AxisListType.X, mybir.AluOpType.add)
                nc.vector.tensor_tensor(
                    hs[:], hs[:],
                    bm_sb[:, bmi, :].unsqueeze(1).broadcast_to([BLK, NBLK, D]),
                    mybir.AluOpType.add)
                ho = hpool.tile([BLK, NBLK, D], dt.float32, tag="hs1")
                nc.scalar.activation(ho[:], hs[:],
                                     mybir.ActivationFunctionType.Copy,
                                     scale=0.125)
                nc.sync.dma_start(
                    out_dram[:, :].rearrange("(b p) d -> p b d", p=BLK), ho[:])

            # ---------------- layer 1 ----------------
            for g, (sT, dT, tab, acc, first) in enumerate([
                    (xAT, xLB, tabs[0], accB, True),
                    (xBT, xLA, tabs[1], accA, True),
                    (xAT, xLA, tabs[0], accA, False)]):
                wt, ks, kp = load_w(w1, g, FIN)
                wrt, ks2, kp2 = load_wr(wr1, g, FIN)
                projection(sT, None, wt, ks, kp, tab, NT1)
                er_sb = er_projection(dT, None, wrt, ks2, kp2)
                conv(g, tab, er_sb, acc, first)

            hTa = hstage(accA, 0, True, hT_loc[0])
            hTb = hstage(accB, 1, True, hT_loc[1])
            import concourse.mybir as _mb
            for s in range(2):
                nc.gpsimd.collective_compute(
                    "AllGather", _mb.AluOpType.bypass,
                    replica_groups=[list(range(NCORE))],
                    ins=[hT_loc[s][:, :]], outs=[hT_full[s][:, :]])

            # ---------------- layer 2 ----------------
            for g, (sFD, dSB, tab, acc, first) in enumerate([
                    (hT_full[0], hTb, tabs[1], accB, True),
                    (hT_full[1], hTa, tabs[0], accA, True),
                    (hT_full[0], hTa, tabs[1], accA, False)]):
                wt, ks, kp = load_w(w2, g, D)
                wrt, ks2, kp2 = load_wr(wr2, g, D)
                projection(None, sFD, wt, ks, kp, tab, NT2)
                er_sb = er_projection(None, dSB, wrt, ks2, kp2)
                conv(g, tab, er_sb, acc, first)

            ostage(accA, 2, oA)
            ostage(accB, 3, oB)

    nc.compile()
    return nc


_CACHE = {}


def _prep_inputs(inputs):
    f32 = np.float32
    xA = np.asarray(inputs["xA"], f32)
    xB = np.asarray(inputs["xB"], f32)
    W1 = np.asarray(inputs["W1"], f32)
    al1 = np.asarray(inputs["al1"], f32)
    ar1 = np.asarray(inputs["ar1"], f32)
    b1 = np.asarray(inputs["b1"], f32)
    W2 = np.asarray(inputs["W2"], f32)
    al2 = np.asarray(inputs["al2"], f32)
    ar2 = np.asarray(inputs["ar2"], f32)
    b2 = np.asarray(inputs["b2"], f32)
    uv = [(np.asarray(inputs["u0"]), np.asarray(inputs["v0"])),
          (np.asarray(inputs["u1"]), np.asarray(inputs["v1"])),
          (np.asarray(inputs["u2"]), np.asarray(inputs["v2"]))]

    graphs = [_prep_graph(u, v) for u, v in uv]
    KS = tuple(g[0] for g in graphs)

    def bf(x):
        return np.ascontiguousarray(x.astype(F16))

    xATn = np.zeros((FIN, NP1), f32)
    xATn[:, :N] = xA.T
    xBTn = np.zeros((FIN, NP1), f32)
    xBTn[:, :N] = xB.T
    w1n = np.stack([np.concatenate([W1[e], _fold(W1[e], al1[e])], 1)
                    for e in range(3)])
    wr1n = np.stack([_fold(W1[e], ar1[e]) for e in range(3)])
    w2n = np.stack([np.concatenate([W2[e], _fold(W2[e], al2[e])], 1)
                    for e in range(3)]) / 8.0
    wr2n = np.stack([_fold(W2[e], ar2[e]) for e in range(3)]) / 8.0
    bmn = np.zeros((4, BLK, D), f32)
    bmn[0] = 8.0 * (b1[1] + b1[2]).reshape(H, D).mean(0)
    bmn[1] = 8.0 * b1[0].reshape(H, D).mean(0)
    bmn[2] = 8.0 * (b2[1] + b2[2]).reshape(H, D).mean(0)
    bmn[3] = 8.0 * b2[0].reshape(H, D).mean(0)

    shared = dict(xAT=bf(xATn), xBT=bf(xBTn), w1=bf(w1n), wr1=bf(wr1n),
                  w2=bf(w2n), wr2=bf(wr2n), bm=bmn,
                  ident=np.eye(BLK).astype(F16))

    in_maps = []
    for c in range(NCORE):
        m = dict(shared)
        xla = np.zeros((FIN, DPC), f32)
        xlb = np.zeros((FIN, DPC), f32)
        lo = c * DPC
        hi = min(N, lo + DPC)
        if hi > lo:
            xla[:, :hi - lo] = xA.T[:, lo:hi]
            xlb[:, :hi - lo] = xB.T[:, lo:hi]
        m["xLA"] = bf(xla)
        m["xLB"] = bf(xlb)
        for g in range(3):
            cd = graphs[g][1][c]
            m[f"u16_{g}"] = cd["u16"]
            m[f"p01_{g}"] = cd["p01"]
            m[f"p01t_{g}"] = cd["p01t"]
        in_maps.append(m)
    return KS, in_maps


LAST_EXEC_NS = None


def kernel(**inputs):
    import os
    import time
    from concourse.bass_utils import run_bass_kernel_spmd

    global LAST_EXEC_NS
    KS, in_maps = _prep_inputs(inputs)
    if KS not in _CACHE:
        _CACHE[KS] = _build_program(list(KS))
    nc = _CACHE[KS]

    res = run_bass_kernel_spmd(nc, in_maps, list(range(NCORE)))
    if os.environ.get("BASS_BENCH"):
        # warm timing: jit/lowering cached after first call; wall includes
        # host<->device transfers, so report the best of a few runs
        best = None
        for _ in range(int(os.environ.get("BASS_BENCH_ITERS", "3"))):
            t0 = time.perf_counter()
            run_bass_kernel_spmd(nc, in_maps, list(range(NCORE)))
            dt_s = time.perf_counter() - t0
            best = dt_s if best is None else min(best, dt_s)
        LAST_EXEC_NS = int(best * 1e9)
        try:
            r2 = run_bass_kernel_spmd(nc, in_maps, list(range(NCORE)),
                                      trace=True)
            if r2.exec_time_ns:
                LAST_EXEC_NS = int(r2.exec_time_ns)
        except Exception as e:
            print("trace unavailable:", type(e).__name__, str(e)[:120])
    f32 = np.float32
    outA = np.zeros((N, D), f32)
    outB = np.zeros((N, D), f32)
    for c in range(NCORE):
        lo = c * DPC
        hi = min(N, lo + DPC)
        outA[lo:hi] = np.asarray(res.results[c]["oA"], f32).reshape(DPC, D)[:hi - lo]
        outB[lo:hi] = np.asarray(res.results[c]["oB"], f32).reshape(DPC, D)[:hi - lo]
    return np.stack([outA, outB]).astype(np.float32)
